# revision 25
# baseline (speedup 1.0000x reference)
"""Trainium2 Bass kernel for nn_LsunIntermediateRotation2dLayer.

Computation: X [64, 256, 256, 16] fp32; per spatial block (r, c) an 8x8
orthonormal matrix R (28 cascaded Givens rotations + mu row signs) is applied
as R^T to channels 8:16; channels 0:8 pass through.

Sharding: data-parallel over rows r — 8 cores x 32 rows each (angles/mus
shard with blocks). Each core runs an identical Bass program on its slice.

Design notes (correctness gate is 2e-2 so bf16 matmul is fine, err ~4e-3):
  - lead-in entirely on the scalar engine: S = Sin(A), C = Sin(A + pi/2)
    via the activation bias (|angles| <= ~0.6 so no range wrap), sign-packed
    sin table via activation scale=-1
  - Givens cascade: 4 DVE/GpSimd ops per rotation on a row-pair view
        P = [rt; rb]*c,  Q = [-s*rb ; +s*rt],  [rt'; rb'] = P + Q
    split in two u-chunks: chunk 1 (u 0..31, rows 0..15) runs in the prefix
    on vector+gpsimd; chunk 2 (u 32..63) runs gpsimd-only, overlapped with
    main-loop iterations 0..15 (gpsimd is otherwise idle in the main loop);
    each chunk gets its own bf16 relayout + xbar-transpose dance into R2
  - main loop: per-iter DMA count kept at 2 (1 load pair... 2 loads,
    2 stores) with the [(h,n),(c,j)] -> [(g,j),o,(h,n)] transpose done as 8
    PE transposes into PSUM + one vector copy, so HWDGE completion
    semaphore lanes don't serialize unrelated transfers
  - engine assignment: vector = cast + YS copy + both bd mask-scatters,
    scalar = PSUM drain + store triggers, sync = load triggers,
    tensor = transposes + matmuls, gpsimd = overlapped cascade chunk 2
"""
import sys

if '/opt/trn_rl_repo' not in sys.path:
    sys.path.insert(0, '/opt/trn_rl_repo')

import math

import numpy as np

N_CORES = 8
NSAMP, NROWS, NCOLS, NCH = 64, 256, 256, 16
RR = NROWS // N_CORES          # 32 rows per core
NBLK = RR * NCOLS              # 8192 blocks per core
NU = NBLK // 128               # 64 partition-tile groups
PS = 8
NANG = 28

_CACHE = {}


def _build_nc(rr_count=RR):
    import concourse.bass as bass
    import concourse.tile as tile
    from concourse import bacc, mybir

    nblk = rr_count * NCOLS
    nu = nblk // 128
    nuq = nu // 16

    f32 = mybir.dt.float32
    bf16 = mybir.dt.bfloat16
    mult = mybir.AluOpType.mult
    add = mybir.AluOpType.add
    Sin = mybir.ActivationFunctionType.Sin
    Copy = mybir.ActivationFunctionType.Copy

    NC2 = nu // 2                  # chunk size (32): chunk1 u<NC2, chunk2 rest

    nc = bacc.Bacc("TRN2", target_bir_lowering=False)
    X_d = nc.declare_dram_parameter("X", [NSAMP, rr_count, NCOLS, NCH], f32, isOutput=False)
    ang_d = nc.declare_dram_parameter("angles", [nblk, NANG], f32, isOutput=False)
    mus_d = nc.declare_dram_parameter("mus", [nblk, PS], f32, isOutput=False)
    out_d = nc.declare_dram_parameter("out", [NSAMP, rr_count, NCOLS, NCH], f32, isOutput=True)
    mask_d = nc.declare_dram_parameter("mask", [128, 128], bf16, isOutput=False)
    ident_d = nc.declare_dram_parameter("ident", [128, 128], bf16, isOutput=False)

    with tile.TileContext(nc) as tc:
        with (
            tc.tile_pool(name="rkeep", bufs=1) as rk,
            tc.tile_pool(name="rbuild", bufs=1) as rp,
            tc.tile_pool(name="io", bufs=8) as iop,
            tc.tile_pool(name="stage", bufs=4) as stp,
            tc.tile_pool(name="bdp", bufs=3) as bdp,
            tc.tile_pool(name="psum", bufs=3, space="PSUM") as psp,
            tc.tile_pool(name="psumt", bufs=2, space="PSUM") as pst,
        ):
            # ---------------- R build lead-in ----------------
            A = rp.tile([128, nu, NANG], f32, tag="A")
            MU = rp.tile([128, nu, PS], f32, tag="MU")
            nc.sync.dma_start(A[:], ang_d[:].rearrange("(u p) k -> p u k", p=128))
            nc.sync.dma_start(MU[:], mus_d[:].rearrange("(u p) k -> p u k", p=128))

            CPI = rk.tile([128, 1], f32, tag="CPI")
            nc.vector.memset(CPI[:], math.pi / 2)
            nc.const_aps.aps[(f32, math.pi / 2)] = CPI[:]
            CM1 = rk.tile([128, 1], f32, tag="CM1")
            nc.vector.memset(CM1[:], -1.0)
            nc.const_aps.aps[(f32, -1.0)] = CM1[:]

            S = rp.tile([128, nu, NANG], f32, tag="S")
            C = rp.tile([128, nu, NANG], f32, tag="C")
            SP = rp.tile([128, nu, 2, NANG], f32, tag="SP")
            nc.scalar.activation(S[:], A[:], Sin)
            nc.scalar.activation(C[:], A[:], Sin, bias=math.pi / 2)
            nc.scalar.activation(SP[:, :, 0], S[:], Copy, scale=-1.0)
            nc.scalar.activation(SP[:, :, 1], S[:], Copy)

            MASKt = rk.tile([128, 128], bf16, tag="MASK")
            nc.sync.dma_start(MASKt[:], mask_d[:])
            maskb = MASKt[:].rearrange("p (g i) -> p g i", g=16)
            IDT = rk.tile([128, 128], bf16, tag="IDT")
            nc.sync.dma_start(IDT[:], ident_d[:])

            R = rp.tile([128, nu, PS, PS], f32, tag="R")
            R2 = rk.tile([128, nuq * 8, 128], bf16, tag="R2")
            r2v = R2[:].rearrange(
                "p (uq o) (u16 i) -> p uq o u16 i", uq=nuq, u16=16)

            def cascade(eng, u0, un, tag):
                """Run the 4-op Givens cascade + mu/bf16 downcast for
                u range [u0, u0+un) on one engine; returns the Rb tile."""
                for j in range(PS):
                    eng.memset(R[:, u0:u0 + un, j, j], 1.0)
                P = rp.tile([128, un, 2, PS], f32, tag=f"P_{tag}")
                Q = rp.tile([128, un, 2, PS], f32, tag=f"Q_{tag}")
                Ru = R[:, u0:u0 + un]
                Cs = C[:, u0:u0 + un]
                SPs = SP[:, u0:u0 + un]
                k = 0
                for t in range(PS - 1):
                    for b in range(t + 1, PS):
                        pair = Ru[:, :, t:b + 1:(b - t), :]       # rows (t, b)
                        Cb = Cs[:, :, k:k + 1].unsqueeze(2).broadcast_to(
                            (128, un, 2, PS))
                        SPn = SPs[:, :, 0, k:k + 1].broadcast_to((128, un, PS))
                        SPp = SPs[:, :, 1, k:k + 1].broadcast_to((128, un, PS))
                        eng.tensor_tensor(out=P[:], in0=pair, in1=Cb, op=mult)
                        eng.tensor_tensor(out=Q[:, :, 0], in0=Ru[:, :, b, :],
                                          in1=SPn, op=mult)
                        eng.tensor_tensor(out=Q[:, :, 1], in0=Ru[:, :, t, :],
                                          in1=SPp, op=mult)
                        eng.tensor_tensor(out=pair, in0=P[:], in1=Q[:], op=add)
                        k += 1

            def mu_cast(eng, Rbt, cb, u0, un):
                # row signs fused with bf16 downcast into (j, u, i) layout
                vb = Rbt[:].transpose([0, 2, 1, 3])  # [128, NC2, j, i] view
                eng.tensor_tensor(
                    out=vb[:, u0 - cb * NC2:u0 - cb * NC2 + un],
                    in0=R[:, u0:u0 + un],
                    in1=MU[:, u0:u0 + un].unsqueeze(3).broadcast_to(
                        (128, un, PS, PS)), op=mult)

            def dance(cb, Rbt):
                """Chunk cb's double-transpose into R2[:, cb*16:(cb+1)*16]."""
                o1 = rp.tile([128, PS * 2, 128], bf16, tag=f"o1_{cb}")
                nc.sync.dma_start(o1[:], Rbt[:], transpose=True)
                tmp = rp.tile([128, 2, 8, 16, PS], bf16, tag=f"tmp_{cb}")
                o1v = o1[:].rearrange("p (j uq) (o g) -> p j uq o g", j=PS, o=8)
                for uql in range(2):
                    nc.vector.tensor_copy(tmp[:, uql],
                                          o1v[:, :, uql].transpose([0, 2, 3, 1]))
                nc.sync.dma_start(R2[:, cb * 16:(cb + 1) * 16, :], tmp[:],
                                  transpose=True)

            # chunk 1 (u 0..NC2): vector-heavy split, runs in the prefix
            UV1 = (NC2 * 11) // 16      # 22 on vector, 10 on gpsimd
            nc.vector.memset(R[:, 0:UV1], 0.0)
            nc.gpsimd.memset(R[:, UV1:], 0.0)
            Rb1 = rp.tile([128, PS, NC2, PS], bf16, tag="Rb1")
            cascade(nc.vector, 0, UV1, "v1")
            cascade(nc.gpsimd, UV1, NC2 - UV1, "g1")
            mu_cast(nc.vector, Rb1, 0, 0, UV1)
            mu_cast(nc.gpsimd, Rb1, 0, UV1, NC2 - UV1)
            dance(0, Rb1)

            # chunk 2 (u NC2..nu): gpsimd only — overlaps main iters 0..15
            Rb2 = rp.tile([128, PS, NC2, PS], bf16, tag="Rb2")
            cascade(nc.gpsimd, NC2, nu - NC2, "g2")
            mu_cast(nc.gpsimd, Rb2, 1, NC2, nu - NC2)

            # ---------------- main loop ----------------
            def emit_iter(rr):
                T0 = iop.tile([128, 128, NCH], f32, tag="T0")
                for h in range(2):
                    nc.sync.dma_start(
                        T0[h * 64:(h + 1) * 64, :, :],
                        X_d[:, rr, h * 128:(h + 1) * 128, :])

                # rotation channels -> bf16 (vector)
                Ab = stp.tile([128, 128, PS], bf16, tag="Ab")
                nc.vector.tensor_copy(Ab[:], T0[:, :, 8:16])

                # transpose [(h,n), (c,j)] -> [(g,j), o, (h,n)]: 8 PE
                # transposes into PSUM, then PSUM -> SBUF on vector
                abf = Ab[:].rearrange("p c j -> p (c j)")
                ysp = pst.tile([128, 8, 128], bf16, tag="ysp")
                for q in range(8):
                    nc.tensor.transpose(ysp[:, q, :],
                                        abf[:, q * 128:(q + 1) * 128], IDT[:])
                YS = stp.tile([128, 8, 128], bf16, tag="YS")
                nc.vector.tensor_copy(YS[:], ysp[:])

                # block-diag weights: bd[8g+j, o, h, 8g'+i] =
                #   mask[g==g'] * R2[8g+j, (Uq, o), (U16(h), i)]
                bd = bdp.tile([128, 8, 2, 128], bf16, tag="bd")
                uq, u16 = (2 * rr) // 16, (2 * rr) % 16
                for h in range(2):
                    in0 = (r2v[:, uq, :, u16 + h, :]
                           .unsqueeze(2)
                           .broadcast_to((128, 8, 16, PS)))
                    in1 = maskb.unsqueeze(1).broadcast_to((128, 8, 16, PS))
                    nc.vector.tensor_tensor(
                        out=bd[:, :, h, :].rearrange(
                            "p o (g i) -> p o g i", g=16),
                        in0=in0, in1=in1, op=mult)

                ps = psp.tile([128, 8, 128], f32, tag="ps")
                for o in range(8):
                    for h in range(2):
                        m_sl = slice(h * 64, h * 64 + 64)
                        nc.tensor.matmul(ps[m_sl, o, :], YS[:, o, m_sl],
                                         bd[:, o, h, :], start=True, stop=True)

                # drain PSUM into T0's rotation-channel slots (scalar)
                t0v = T0[:].rearrange("p (o g) ch -> p o g ch", g=16)
                psv = ps[:].rearrange("p o (g i) -> p o g i", g=16)
                nc.scalar.activation(t0v[:, :, :, 8:16], psv[:], Copy)

                for h in range(2):
                    nc.scalar.dma_start(
                        out_d[:, rr, h * 128:(h + 1) * 128, :],
                        T0[h * 64:(h + 1) * 64, :, :])

            half = rr_count // 2
            for rr in range(half):
                emit_iter(rr)
            # chunk 2's dance, emitted after iter 15 so its xbar waits don't
            # block the sync/vector streams during iters 0..15
            dance(1, Rb2)
            for rr in range(half, rr_count):
                emit_iter(rr)

    nc.finalize()
    return nc


def _get_nc():
    if "nc" not in _CACHE:
        _CACHE["nc"] = _build_nc()
    return _CACHE["nc"]


def block_diag_mask():
    import ml_dtypes
    m = np.kron(np.eye(16, dtype=np.float32), np.ones((8, 8), dtype=np.float32))
    return np.ascontiguousarray(m.astype(ml_dtypes.bfloat16))


def identity128():
    import ml_dtypes
    return np.ascontiguousarray(np.eye(128, dtype=np.float32).astype(ml_dtypes.bfloat16))


def make_in_maps(X, angles, mus):
    mask = block_diag_mask()
    ident = identity128()
    in_maps = []
    for c in range(N_CORES):
        in_maps.append({
            "X": np.ascontiguousarray(X[:, c * RR:(c + 1) * RR]),
            "angles": np.ascontiguousarray(angles[c * NBLK:(c + 1) * NBLK]),
            "mus": np.ascontiguousarray(mus[c * NBLK:(c + 1) * NBLK]),
            "mask": mask,
            "ident": ident,
        })
    return in_maps


def kernel(X, angles, mus):
    from concourse.bass_utils import run_bass_kernel_spmd

    X = np.ascontiguousarray(X, dtype=np.float32)
    angles = np.ascontiguousarray(angles, dtype=np.float32)
    mus = np.ascontiguousarray(mus, dtype=np.float32)

    nc = _get_nc()
    in_maps = make_in_maps(X, angles, mus)
    res = run_bass_kernel_spmd(nc, in_maps, list(range(N_CORES)))
    out = np.concatenate([res.results[c]["out"] for c in range(N_CORES)], axis=1)
    return out


# revision 28
# speedup vs baseline: 1.0018x; 1.0018x over previous
"""Trainium2 Bass kernel for nn_LsunIntermediateRotation2dLayer.

Computation: X [64, 256, 256, 16] fp32; per spatial block (r, c) an 8x8
orthonormal matrix R (28 cascaded Givens rotations + mu row signs) is applied
as R^T to channels 8:16; channels 0:8 pass through.

Sharding: data-parallel over rows r — 8 cores x 32 rows each (angles/mus
shard with blocks). Each core runs an identical Bass program on its slice.

Design notes (correctness gate is 2e-2 so bf16 matmul is fine, err ~4e-3):
  - lead-in entirely on the scalar engine: S = Sin(A), C = Sin(A + pi/2)
    via the activation bias (|angles| <= ~0.6 so no range wrap), sign-packed
    sin table via activation scale=-1
  - Givens cascade: 4 DVE/GpSimd ops per rotation on a row-pair view
        P = [rt; rb]*c,  Q = [-s*rb ; +s*rt],  [rt'; rb'] = P + Q
    split in two u-chunks: chunk 1 (u 0..31, rows 0..15) runs in the prefix
    on vector+gpsimd; chunk 2 (u 32..63) runs gpsimd-only, overlapped with
    main-loop iterations 0..15 (gpsimd is otherwise idle in the main loop);
    each chunk gets its own bf16 relayout + xbar-transpose dance into R2
  - main loop: per-iter DMA count kept at 2 (1 load pair... 2 loads,
    2 stores) with the [(h,n),(c,j)] -> [(g,j),o,(h,n)] transpose done as 8
    PE transposes into PSUM + one vector copy, so HWDGE completion
    semaphore lanes don't serialize unrelated transfers
  - engine assignment: vector = cast + YS copy + both bd mask-scatters,
    scalar = PSUM drain + store triggers, sync = load triggers,
    tensor = transposes + matmuls, gpsimd = overlapped cascade chunk 2
"""
import sys

if '/opt/trn_rl_repo' not in sys.path:
    sys.path.insert(0, '/opt/trn_rl_repo')

import math

import numpy as np

N_CORES = 8
NSAMP, NROWS, NCOLS, NCH = 64, 256, 256, 16
RR = NROWS // N_CORES          # 32 rows per core
NBLK = RR * NCOLS              # 8192 blocks per core
NU = NBLK // 128               # 64 partition-tile groups
PS = 8
NANG = 28

_CACHE = {}


def _build_nc(rr_count=RR):
    import concourse.bass as bass
    import concourse.tile as tile
    from concourse import bacc, mybir

    nblk = rr_count * NCOLS
    nu = nblk // 128
    nuq = nu // 16

    f32 = mybir.dt.float32
    bf16 = mybir.dt.bfloat16
    mult = mybir.AluOpType.mult
    add = mybir.AluOpType.add
    Sin = mybir.ActivationFunctionType.Sin
    Copy = mybir.ActivationFunctionType.Copy

    NC2 = nu // 2                  # chunk size (32): chunk1 u<NC2, chunk2 rest

    nc = bacc.Bacc("TRN2", target_bir_lowering=False)
    X_d = nc.declare_dram_parameter("X", [NSAMP, rr_count, NCOLS, NCH], f32, isOutput=False)
    ang_d = nc.declare_dram_parameter("angles", [nblk, NANG], f32, isOutput=False)
    mus_d = nc.declare_dram_parameter("mus", [nblk, PS], f32, isOutput=False)
    out_d = nc.declare_dram_parameter("out", [NSAMP, rr_count, NCOLS, NCH], f32, isOutput=True)
    mask_d = nc.declare_dram_parameter("mask", [128, 128], bf16, isOutput=False)
    ident_d = nc.declare_dram_parameter("ident", [128, 128], bf16, isOutput=False)

    with tile.TileContext(nc) as tc:
        with (
            tc.tile_pool(name="rkeep", bufs=1) as rk,
            tc.tile_pool(name="rbuild", bufs=1) as rp,
            tc.tile_pool(name="io", bufs=8) as iop,
            tc.tile_pool(name="stage", bufs=4) as stp,
            tc.tile_pool(name="bdp", bufs=3) as bdp,
            tc.tile_pool(name="psum", bufs=3, space="PSUM") as psp,
            tc.tile_pool(name="psumt", bufs=2, space="PSUM") as pst,
        ):
            # ---------------- R build lead-in ----------------
            A = rp.tile([128, nu, NANG], f32, tag="A")
            MU = rp.tile([128, nu, PS], f32, tag="MU")
            nc.sync.dma_start(A[:], ang_d[:].rearrange("(u p) k -> p u k", p=128))
            nc.sync.dma_start(MU[:], mus_d[:].rearrange("(u p) k -> p u k", p=128))

            CPI = rk.tile([128, 1], f32, tag="CPI")
            nc.vector.memset(CPI[:], math.pi / 2)
            nc.const_aps.aps[(f32, math.pi / 2)] = CPI[:]
            CM1 = rk.tile([128, 1], f32, tag="CM1")
            nc.vector.memset(CM1[:], -1.0)
            nc.const_aps.aps[(f32, -1.0)] = CM1[:]

            S = rp.tile([128, nu, NANG], f32, tag="S")
            C = rp.tile([128, nu, NANG], f32, tag="C")
            SP = rp.tile([128, nu, 2, NANG], f32, tag="SP")
            nc.scalar.activation(S[:], A[:], Sin)
            nc.scalar.activation(C[:], A[:], Sin, bias=math.pi / 2)
            nc.scalar.activation(SP[:, :, 0], S[:], Copy, scale=-1.0)
            nc.scalar.activation(SP[:, :, 1], S[:], Copy)

            MASKt = rk.tile([128, 128], bf16, tag="MASK")
            nc.sync.dma_start(MASKt[:], mask_d[:])
            maskb = MASKt[:].rearrange("p (g i) -> p g i", g=16)
            IDT = rk.tile([128, 128], bf16, tag="IDT")
            nc.sync.dma_start(IDT[:], ident_d[:])

            R = rp.tile([128, nu, PS, PS], f32, tag="R")
            R2 = rk.tile([128, nuq * 8, 128], bf16, tag="R2")
            r2v = R2[:].rearrange(
                "p (uq o) (u16 i) -> p uq o u16 i", uq=nuq, u16=16)

            def cascade(eng, u0, un, tag):
                """Run the 4-op Givens cascade + mu/bf16 downcast for
                u range [u0, u0+un) on one engine; returns the Rb tile."""
                for j in range(PS):
                    eng.memset(R[:, u0:u0 + un, j, j], 1.0)
                P = rp.tile([128, un, 2, PS], f32, tag=f"P_{tag}")
                Q = rp.tile([128, un, 2, PS], f32, tag=f"Q_{tag}")
                Ru = R[:, u0:u0 + un]
                Cs = C[:, u0:u0 + un]
                SPs = SP[:, u0:u0 + un]
                k = 0
                for t in range(PS - 1):
                    for b in range(t + 1, PS):
                        pair = Ru[:, :, t:b + 1:(b - t), :]       # rows (t, b)
                        Cb = Cs[:, :, k:k + 1].unsqueeze(2).broadcast_to(
                            (128, un, 2, PS))
                        SPn = SPs[:, :, 0, k:k + 1].broadcast_to((128, un, PS))
                        SPp = SPs[:, :, 1, k:k + 1].broadcast_to((128, un, PS))
                        eng.tensor_tensor(out=P[:], in0=pair, in1=Cb, op=mult)
                        eng.tensor_tensor(out=Q[:, :, 0], in0=Ru[:, :, b, :],
                                          in1=SPn, op=mult)
                        eng.tensor_tensor(out=Q[:, :, 1], in0=Ru[:, :, t, :],
                                          in1=SPp, op=mult)
                        eng.tensor_tensor(out=pair, in0=P[:], in1=Q[:], op=add)
                        k += 1

            def mu_cast(eng, Rbt, cb, u0, un):
                # row signs fused with bf16 downcast into (j, u, i) layout
                vb = Rbt[:].transpose([0, 2, 1, 3])  # [128, u, j, i] view
                eng.tensor_tensor(
                    out=vb[:, u0:u0 + un],
                    in0=R[:, u0:u0 + un],
                    in1=MU[:, u0:u0 + un].unsqueeze(3).broadcast_to(
                        (128, un, PS, PS)), op=mult)

            def dance(Rbt):
                """Double-transpose [blk, (j,U,i)] -> [(g,j), (Uq o), (U16 i)]."""
                o1 = rp.tile([128, PS * nuq, 128], bf16, tag="o1")
                nc.sync.dma_start(o1[:], Rbt[:], transpose=True)
                tmp = rp.tile([128, nuq, 8, 16, PS], bf16, tag="tmp")
                o1v = o1[:].rearrange("p (j uq) (o g) -> p j uq o g", j=PS, o=8)
                for uql in range(nuq):
                    nc.vector.tensor_copy(tmp[:, uql],
                                          o1v[:, :, uql].transpose([0, 2, 3, 1]))
                nc.sync.dma_start(R2[:], tmp[:], transpose=True)

            # cascade over all u, split vector-heavy; single dance
            UV1 = (nu * 11) // 16       # 44 on vector, 20 on gpsimd
            nc.vector.memset(R[:, 0:UV1], 0.0)
            nc.gpsimd.memset(R[:, UV1:], 0.0)
            Rb1 = rp.tile([128, PS, nu, PS], bf16, tag="Rb1")
            cascade(nc.vector, 0, UV1, "v1")
            cascade(nc.gpsimd, UV1, nu - UV1, "g1")
            mu_cast(nc.vector, Rb1, 0, 0, UV1)
            mu_cast(nc.gpsimd, Rb1, 0, UV1, nu - UV1)
            dance(Rb1)

            # ---------------- main loop ----------------
            def emit_iter(rr):
                T0 = iop.tile([128, 128, NCH], f32, tag="T0")
                for h in range(2):
                    nc.sync.dma_start(
                        T0[h * 64:(h + 1) * 64, :, :],
                        X_d[:, rr, h * 128:(h + 1) * 128, :])

                # rotation channels -> bf16 (vector)
                Ab = stp.tile([128, 128, PS], bf16, tag="Ab")
                nc.vector.tensor_copy(Ab[:], T0[:, :, 8:16])

                # transpose [(h,n), (c,j)] -> [(g,j), o, (h,n)]: 8 PE
                # transposes into PSUM, then PSUM -> SBUF on vector
                abf = Ab[:].rearrange("p c j -> p (c j)")
                ysp = pst.tile([128, 8, 128], bf16, tag="ysp")
                for q in range(8):
                    nc.tensor.transpose(ysp[:, q, :],
                                        abf[:, q * 128:(q + 1) * 128], IDT[:])
                YS = stp.tile([128, 8, 128], bf16, tag="YS")
                nc.vector.tensor_copy(YS[:], ysp[:])

                # block-diag weights: bd[8g+j, o, h, 8g'+i] =
                #   mask[g==g'] * R2[8g+j, (Uq, o), (U16(h), i)]
                bd = bdp.tile([128, 8, 2, 128], bf16, tag="bd")
                uq, u16 = (2 * rr) // 16, (2 * rr) % 16
                for h in range(2):
                    in0 = (r2v[:, uq, :, u16 + h, :]
                           .unsqueeze(2)
                           .broadcast_to((128, 8, 16, PS)))
                    in1 = maskb.unsqueeze(1).broadcast_to((128, 8, 16, PS))
                    nc.vector.tensor_tensor(
                        out=bd[:, :, h, :].rearrange(
                            "p o (g i) -> p o g i", g=16),
                        in0=in0, in1=in1, op=mult)

                ps = psp.tile([128, 8, 128], f32, tag="ps")
                for o in range(8):
                    for h in range(2):
                        m_sl = slice(h * 64, h * 64 + 64)
                        nc.tensor.matmul(ps[m_sl, o, :], YS[:, o, m_sl],
                                         bd[:, o, h, :], start=True, stop=True)

                # drain PSUM into T0's rotation-channel slots (scalar)
                t0v = T0[:].rearrange("p (o g) ch -> p o g ch", g=16)
                psv = ps[:].rearrange("p o (g i) -> p o g i", g=16)
                nc.scalar.activation(t0v[:, :, :, 8:16], psv[:], Copy)

                for h in range(2):
                    nc.scalar.dma_start(
                        out_d[:, rr, h * 128:(h + 1) * 128, :],
                        T0[h * 64:(h + 1) * 64, :, :])

            for rr in range(rr_count):
                emit_iter(rr)

    nc.finalize()
    return nc


def _get_nc():
    if "nc" not in _CACHE:
        _CACHE["nc"] = _build_nc()
    return _CACHE["nc"]


def block_diag_mask():
    import ml_dtypes
    m = np.kron(np.eye(16, dtype=np.float32), np.ones((8, 8), dtype=np.float32))
    return np.ascontiguousarray(m.astype(ml_dtypes.bfloat16))


def identity128():
    import ml_dtypes
    return np.ascontiguousarray(np.eye(128, dtype=np.float32).astype(ml_dtypes.bfloat16))


def make_in_maps(X, angles, mus):
    mask = block_diag_mask()
    ident = identity128()
    in_maps = []
    for c in range(N_CORES):
        in_maps.append({
            "X": np.ascontiguousarray(X[:, c * RR:(c + 1) * RR]),
            "angles": np.ascontiguousarray(angles[c * NBLK:(c + 1) * NBLK]),
            "mus": np.ascontiguousarray(mus[c * NBLK:(c + 1) * NBLK]),
            "mask": mask,
            "ident": ident,
        })
    return in_maps


def kernel(X, angles, mus):
    from concourse.bass_utils import run_bass_kernel_spmd

    X = np.ascontiguousarray(X, dtype=np.float32)
    angles = np.ascontiguousarray(angles, dtype=np.float32)
    mus = np.ascontiguousarray(mus, dtype=np.float32)

    nc = _get_nc()
    in_maps = make_in_maps(X, angles, mus)
    res = run_bass_kernel_spmd(nc, in_maps, list(range(N_CORES)))
    out = np.concatenate([res.results[c]["out"] for c in range(N_CORES)], axis=1)
    return out


# revision 36
# speedup vs baseline: 1.0935x; 1.0916x over previous
"""Trainium2 Bass kernel for nn_LsunIntermediateRotation2dLayer.

Computation: X [64, 256, 256, 16] fp32; per spatial block (r, c) an 8x8
orthonormal matrix R (28 cascaded Givens rotations + mu row signs) is applied
as R^T to channels 8:16; channels 0:8 pass through.

Sharding: data-parallel over rows r — 8 cores x 32 rows each (angles/mus
shard with blocks). Each core runs an identical Bass program on its slice.

Design notes (correctness gate is 2e-2 so bf16 matmul is fine, err ~4e-3):
  - lead-in entirely on the scalar engine: S = Sin(A), C = Sin(A + pi/2)
    via the activation bias (|angles| <= ~0.6 so no range wrap), sign-packed
    sin table via activation scale=-1
  - Givens cascade: 4 DVE/GpSimd ops per rotation on a row-pair view
        P = [rt; rb]*c,  Q = [-s*rb ; +s*rt],  [rt'; rb'] = P + Q
    split in two u-chunks: chunk 1 (u 0..31, rows 0..15) runs in the prefix
    on vector+gpsimd; chunk 2 (u 32..63) runs gpsimd-only, overlapped with
    main-loop iterations 0..15 (gpsimd is otherwise idle in the main loop);
    each chunk gets its own bf16 relayout + xbar-transpose dance into R2
  - main loop: per-iter DMA count kept at 2 (1 load pair... 2 loads,
    2 stores) with the [(h,n),(c,j)] -> [(g,j),o,(h,n)] transpose done as 8
    PE transposes into PSUM + one vector copy, so HWDGE completion
    semaphore lanes don't serialize unrelated transfers
  - engine assignment: vector = cast + YS copy + both bd mask-scatters,
    scalar = PSUM drain + store triggers, sync = load triggers,
    tensor = transposes + matmuls, gpsimd = overlapped cascade chunk 2
"""
import sys

if '/opt/trn_rl_repo' not in sys.path:
    sys.path.insert(0, '/opt/trn_rl_repo')

import math

import numpy as np

N_CORES = 8
NSAMP, NROWS, NCOLS, NCH = 64, 256, 256, 16
RR = NROWS // N_CORES          # 32 rows per core
NBLK = RR * NCOLS              # 8192 blocks per core
NU = NBLK // 128               # 64 partition-tile groups
PS = 8
NANG = 28

_CACHE = {}


def _build_nc(rr_count=RR):
    import concourse.bass as bass
    import concourse.tile as tile
    from concourse import bacc, mybir

    nblk = rr_count * NCOLS
    nu = nblk // 128
    nuq = nu // 16

    f32 = mybir.dt.float32
    bf16 = mybir.dt.bfloat16
    mult = mybir.AluOpType.mult
    add = mybir.AluOpType.add
    Sin = mybir.ActivationFunctionType.Sin
    Copy = mybir.ActivationFunctionType.Copy

    NC2 = nu // 2                  # chunk size (32): chunk1 u<NC2, chunk2 rest

    nc = bacc.Bacc("TRN2", target_bir_lowering=False)
    X_d = nc.declare_dram_parameter("X", [NSAMP, rr_count, NCOLS, NCH], f32, isOutput=False)
    ang_d = nc.declare_dram_parameter("angles", [nblk, NANG], f32, isOutput=False)
    mus_d = nc.declare_dram_parameter("mus", [nblk, PS], f32, isOutput=False)
    out_d = nc.declare_dram_parameter("out", [NSAMP, rr_count, NCOLS, NCH], f32, isOutput=True)
    mask_d = nc.declare_dram_parameter("mask", [128, 128], bf16, isOutput=False)
    ident_d = nc.declare_dram_parameter("ident", [128, 128], bf16, isOutput=False)

    with tile.TileContext(nc) as tc:
        with (
            tc.tile_pool(name="rkeep", bufs=1) as rk,
            tc.tile_pool(name="rbuild", bufs=1) as rp,
            tc.tile_pool(name="io", bufs=8) as iop,
            tc.tile_pool(name="abp", bufs=3) as abp,
            tc.tile_pool(name="ysq", bufs=10) as ysq,
            tc.tile_pool(name="bdp", bufs=3) as bdp,
            tc.tile_pool(name="psum", bufs=3, space="PSUM") as psp,
            tc.tile_pool(name="psumt", bufs=2, space="PSUM") as pst,
        ):
            # ---------------- R build lead-in ----------------
            A = rp.tile([128, nu, NANG], f32, tag="A")
            MU = rp.tile([128, nu, PS], f32, tag="MU")
            nc.sync.dma_start(A[:], ang_d[:].rearrange("(u p) k -> p u k", p=128))
            nc.sync.dma_start(MU[:], mus_d[:].rearrange("(u p) k -> p u k", p=128))

            CPI = rk.tile([128, 1], f32, tag="CPI")
            nc.vector.memset(CPI[:], math.pi / 2)
            nc.const_aps.aps[(f32, math.pi / 2)] = CPI[:]
            CM1 = rk.tile([128, 1], f32, tag="CM1")
            nc.vector.memset(CM1[:], -1.0)
            nc.const_aps.aps[(f32, -1.0)] = CM1[:]

            S = rp.tile([128, nu, NANG], f32, tag="S")
            C = rp.tile([128, nu, NANG], f32, tag="C")
            SP = rp.tile([128, nu, 2, NANG], f32, tag="SP")
            nc.scalar.activation(S[:], A[:], Sin)
            nc.scalar.activation(C[:], A[:], Sin, bias=math.pi / 2)
            nc.scalar.activation(SP[:, :, 0], S[:], Copy, scale=-1.0)
            nc.scalar.activation(SP[:, :, 1], S[:], Copy)

            MASKt = rk.tile([128, 128], bf16, tag="MASK")
            nc.sync.dma_start(MASKt[:], mask_d[:])
            maskb = MASKt[:].rearrange("p (g i) -> p g i", g=16)
            IDT = rk.tile([128, 128], bf16, tag="IDT")
            nc.sync.dma_start(IDT[:], ident_d[:])

            R = rp.tile([128, nu, PS, PS], f32, tag="R")
            R2 = rk.tile([128, nuq * 8, 128], bf16, tag="R2")
            r2v = R2[:].rearrange(
                "p (uq o) (u16 i) -> p uq o u16 i", uq=nuq, u16=16)

            def load_T0(rr):
                T0 = iop.tile([128, 128, NCH], f32, tag="T0")
                for h in range(2):
                    nc.sync.dma_start(
                        T0[h * 64:(h + 1) * 64, :, :],
                        X_d[:, rr, h * 128:(h + 1) * 128, :])
                return T0

            def stage_front(T0, pre):
                """cast (scalar if pre, else vector) + 8 PE transposes into
                PSUM + PSUM->SBUF copy on vector; returns the YS tile."""
                Ab = abp.tile([128, 128, PS], bf16, tag="Ab")
                if pre:
                    nc.scalar.activation(Ab[:], T0[:, :, 8:16], Copy)
                else:
                    nc.vector.tensor_copy(Ab[:], T0[:, :, 8:16])
                abf = Ab[:].rearrange("p c j -> p (c j)")
                ysp = pst.tile([128, 8, 128], bf16, tag="ysp")
                for q in range(8):
                    nc.tensor.transpose(ysp[:, q, :],
                                        abf[:, q * 128:(q + 1) * 128], IDT[:])
                YS = ysq.tile([128, 8, 128], bf16, tag="YS")
                nc.vector.tensor_copy(YS[:], ysp[:])
                return YS

            # pre-stage the first PRE rows: loads + scalar casts + PE
            # transposes run during the cascade; the vector YS copies are
            # emitted here, ahead of the cascade in vector's stream.
            PRE = 8
            pre_T0 = {}
            pre_YS = {}
            for rr in range(PRE):
                pre_T0[rr] = load_T0(rr)
                pre_YS[rr] = stage_front(pre_T0[rr], pre=True)

            def cascade(eng, u0, un, tag):
                """Run the 4-op Givens cascade + mu/bf16 downcast for
                u range [u0, u0+un) on one engine; returns the Rb tile."""
                for j in range(PS):
                    eng.memset(R[:, u0:u0 + un, j, j], 1.0)
                P = rp.tile([128, un, 2, PS], f32, tag=f"P_{tag}")
                Q = rp.tile([128, un, 2, PS], f32, tag=f"Q_{tag}")
                Ru = R[:, u0:u0 + un]
                Cs = C[:, u0:u0 + un]
                SPs = SP[:, u0:u0 + un]
                k = 0
                for t in range(PS - 1):
                    for b in range(t + 1, PS):
                        pair = Ru[:, :, t:b + 1:(b - t), :]       # rows (t, b)
                        Cb = Cs[:, :, k:k + 1].unsqueeze(2).broadcast_to(
                            (128, un, 2, PS))
                        SPn = SPs[:, :, 0, k:k + 1].broadcast_to((128, un, PS))
                        SPp = SPs[:, :, 1, k:k + 1].broadcast_to((128, un, PS))
                        eng.tensor_tensor(out=P[:], in0=pair, in1=Cb, op=mult)
                        eng.tensor_tensor(out=Q[:, :, 0], in0=Ru[:, :, b, :],
                                          in1=SPn, op=mult)
                        eng.tensor_tensor(out=Q[:, :, 1], in0=Ru[:, :, t, :],
                                          in1=SPp, op=mult)
                        eng.tensor_tensor(out=pair, in0=P[:], in1=Q[:], op=add)
                        k += 1

            def mu_cast(eng, Rbt, cb, u0, un):
                # row signs fused with bf16 downcast into (j, u, i) layout
                vb = Rbt[:].transpose([0, 2, 1, 3])  # [128, u, j, i] view
                eng.tensor_tensor(
                    out=vb[:, u0:u0 + un],
                    in0=R[:, u0:u0 + un],
                    in1=MU[:, u0:u0 + un].unsqueeze(3).broadcast_to(
                        (128, un, PS, PS)), op=mult)

            def dance(Rbt):
                """Double-transpose [blk, (j,U,i)] -> [(g,j), (Uq o), (U16 i)]."""
                o1 = rp.tile([128, PS * nuq, 128], bf16, tag="o1")
                nc.sync.dma_start(o1[:], Rbt[:], transpose=True)
                tmp = rp.tile([128, nuq, 8, 16, PS], bf16, tag="tmp")
                o1v = o1[:].rearrange("p (j uq) (o g) -> p j uq o g", j=PS, o=8)
                for uql in range(nuq):
                    nc.vector.tensor_copy(tmp[:, uql],
                                          o1v[:, :, uql].transpose([0, 2, 3, 1]))
                nc.sync.dma_start(R2[:], tmp[:], transpose=True)

            # cascade over all u, split vector-heavy; single dance
            UV1 = (nu * 11) // 16       # 44 on vector, 20 on gpsimd
            nc.vector.memset(R[:, 0:UV1], 0.0)
            nc.gpsimd.memset(R[:, UV1:], 0.0)
            Rb1 = rp.tile([128, PS, nu, PS], bf16, tag="Rb1")
            cascade(nc.vector, 0, UV1, "v1")
            cascade(nc.gpsimd, UV1, nu - UV1, "g1")
            mu_cast(nc.vector, Rb1, 0, 0, UV1)
            mu_cast(nc.gpsimd, Rb1, 0, UV1, nu - UV1)
            dance(Rb1)

            # ---------------- main loop ----------------
            def emit_iter(rr):
                if rr < PRE:
                    T0 = pre_T0[rr]
                    YS = pre_YS[rr]
                else:
                    T0 = load_T0(rr)
                    YS = stage_front(T0, pre=False)

                # block-diag weights: bd[8g+j, o, h, 8g'+i] =
                #   mask[g==g'] * R2[8g+j, (Uq, o), (U16(h), i)]
                bd = bdp.tile([128, 8, 2, 128], bf16, tag="bd")
                uq, u16 = (2 * rr) // 16, (2 * rr) % 16
                for h in range(2):
                    in0 = (r2v[:, uq, :, u16 + h, :]
                           .unsqueeze(2)
                           .broadcast_to((128, 8, 16, PS)))
                    in1 = maskb.unsqueeze(1).broadcast_to((128, 8, 16, PS))
                    nc.vector.tensor_tensor(
                        out=bd[:, :, h, :].rearrange(
                            "p o (g i) -> p o g i", g=16),
                        in0=in0, in1=in1, op=mult)

                ps = psp.tile([128, 8, 128], f32, tag="ps")
                for o in range(8):
                    for h in range(2):
                        m_sl = slice(h * 64, h * 64 + 64)
                        nc.tensor.matmul(ps[m_sl, o, :], YS[:, o, m_sl],
                                         bd[:, o, h, :], start=True, stop=True)

                # drain PSUM into T0's rotation-channel slots (scalar)
                t0v = T0[:].rearrange("p (o g) ch -> p o g ch", g=16)
                psv = ps[:].rearrange("p o (g i) -> p o g i", g=16)
                nc.scalar.activation(t0v[:, :, :, 8:16], psv[:], Copy)

                for h in range(2):
                    nc.scalar.dma_start(
                        out_d[:, rr, h * 128:(h + 1) * 128, :],
                        T0[h * 64:(h + 1) * 64, :, :])

            for rr in range(rr_count):
                emit_iter(rr)

    nc.finalize()
    return nc


def _get_nc():
    if "nc" not in _CACHE:
        _CACHE["nc"] = _build_nc()
    return _CACHE["nc"]


def block_diag_mask():
    import ml_dtypes
    m = np.kron(np.eye(16, dtype=np.float32), np.ones((8, 8), dtype=np.float32))
    return np.ascontiguousarray(m.astype(ml_dtypes.bfloat16))


def identity128():
    import ml_dtypes
    return np.ascontiguousarray(np.eye(128, dtype=np.float32).astype(ml_dtypes.bfloat16))


def make_in_maps(X, angles, mus):
    mask = block_diag_mask()
    ident = identity128()
    in_maps = []
    for c in range(N_CORES):
        in_maps.append({
            "X": np.ascontiguousarray(X[:, c * RR:(c + 1) * RR]),
            "angles": np.ascontiguousarray(angles[c * NBLK:(c + 1) * NBLK]),
            "mus": np.ascontiguousarray(mus[c * NBLK:(c + 1) * NBLK]),
            "mask": mask,
            "ident": ident,
        })
    return in_maps


def kernel(X, angles, mus):
    from concourse.bass_utils import run_bass_kernel_spmd

    X = np.ascontiguousarray(X, dtype=np.float32)
    angles = np.ascontiguousarray(angles, dtype=np.float32)
    mus = np.ascontiguousarray(mus, dtype=np.float32)

    nc = _get_nc()
    in_maps = make_in_maps(X, angles, mus)
    res = run_bass_kernel_spmd(nc, in_maps, list(range(N_CORES)))
    out = np.concatenate([res.results[c]["out"] for c in range(N_CORES)], axis=1)
    return out
